# revision 26
# baseline (speedup 1.0000x reference)
"""Multi-head causal attention (B=4, T=2048, D=1024, H=16) on 8 TRN2 cores.

Tensor-parallel over heads: core c computes heads {2c, 2c+1}. Each core:
  - Q', K', V' feature-major ([feat, tok]) via 512-wide PE matmuls,
  - V' -> token-major V via PE transposes, stored as [ones|v_h0|ones|v_h1] so
    each head's 128-wide PV lhsT slice is [ones(64) | v_h(64)]: the ones
    columns replicate the softmax denominator onto PSUM partitions 0-63
    (the HW custom-DVE reciprocal reads partition base 0 regardless of the
    AP offset, so the denominator must live there),
  - S^T = K'^T Q' tiles [128 k x 512 q], h0/h1 matmuls alternated so the
    64-deep score matmuls run concurrently on PE row groups 0-63/64-127;
    one exp covers each pair's live range (dead gap is never read by PV),
  - exp (no max subtraction: |S|/32 <= ~2), multiplicative 0/1 causal mask
    on diagonal blocks, narrowed to their live query range,
  - PV accumulates [denom replicas | y_h] per psum bank; normalization is a
    DVE reciprocal_approx_fast into SBUF + one partition-offset multiply,
  - w_proj row-slice partial matmuls per query chunk, lag-1 behind PV,
  - partial projections written bf16; host sums the 8 cores and transposes.

Scheduling: the whole emission is software-pipelined. The next batch's QKV
is split into six half-sweeps interleaved into the current batch's attention
(filler for the PE while ACT drains exps — keeps the HAM clock gate at
2.4 GHz); V transposes go in groups of four between dense stretches
(transpose-mode doesn't count as PE activity for HAM); the last batch's
largest score chunk is hoisted into the previous batch and its drain runs
largest-first so the tail ends on the smallest chunk. 250 warmup matmuls
cover the initial x DMA.
"""

import sys

for _p in ("/opt/trn_rl_repo",):
    if _p not in sys.path:
        sys.path.append(_p)

import numpy as np
import ml_dtypes

B, T, D = 4, 2048, 1024
H = 16
HD = D // H
NORM = float(np.sqrt(D))
N_CORES = 8
HEADS_PER_CORE = H // N_CORES          # 2
FPC = HEADS_PER_CORE * HD              # 128 features per core
QC = 512                               # query chunk
NQC = T // QC                          # 4
KB = 128                               # key block
DKC = D // 128                         # 8 contraction chunks over D

_BF16 = ml_dtypes.bfloat16

_cache = {}


def _build():
    import concourse.bacc as bacc
    import concourse.mybir as mybir
    from concourse.tile import TileContext
    from concourse.alu_op_type import AluOpType
    from concourse.masks import make_identity

    f32 = mybir.dt.float32
    bf16 = mybir.dt.bfloat16
    EXP = mybir.ActivationFunctionType.Exp

    nc = bacc.Bacc("TRN2", target_bir_lowering=False, debug=False,
                   num_devices=N_CORES)

    xt = nc.dram_tensor("xt", [B, D, T], bf16, kind="ExternalInput").ap()
    w3 = nc.dram_tensor("w3", [D, 3 * FPC], bf16, kind="ExternalInput").ap()
    wp = nc.dram_tensor("wp", [FPC, D], bf16, kind="ExternalInput").ap()
    masks = nc.dram_tensor("masks", [4, KB, QC], bf16, kind="ExternalInput").ap()
    out = nc.dram_tensor("out", [B, D, T], bf16, kind="ExternalOutput").ap()

    with TileContext(nc) as tc:
        with (
            tc.tile_pool(name="const", bufs=1) as cpool,
            tc.tile_pool(name="xp", bufs=10) as xpool,
            tc.tile_pool(name="qk", bufs=2) as qkpool,
            tc.tile_pool(name="vaug", bufs=24) as vpool,
            tc.tile_pool(name="pt", bufs=40) as ptpool,
            tc.tile_pool(name="y", bufs=4) as ypool,
            tc.tile_pool(name="ot", bufs=3) as otpool,
            tc.tile_pool(name="rec", bufs=2) as recpool,
            tc.tile_pool(name="psA", bufs=2, space="PSUM") as psA,
            tc.tile_pool(name="psY", bufs=2, space="PSUM") as psY,
            tc.tile_pool(name="psO", bufs=2, space="PSUM") as psO,
        ):
            # ---- constants ----
            w3_t = []
            for kc in range(DKC):
                t = cpool.tile([128, 3 * FPC], bf16, tag=f"w3{kc}")
                nc.sync.dma_start(t[:], w3[kc * 128:(kc + 1) * 128, :])
                w3_t.append(t)
            wp_t = cpool.tile([FPC, D], bf16, tag="wp")
            nc.sync.dma_start(wp_t[:], wp[:])
            mask_t = []
            for p in range(4):
                t = cpool.tile([KB, QC], bf16, tag=f"mask{p}")
                nc.sync.dma_start(t[:], masks[p])
                mask_t.append(t)
            ident = cpool.tile([128, 128], bf16, tag="ident")
            make_identity(nc, ident[:])

            # PE warmup during the initial x DMA: keeps the HAM clock-gate
            # busy so real matmuls start at 2.4 GHz.
            psw = psO.tile([128, QC], f32, tag="pso")
            for _ in range(250):
                nc.tensor.matmul(psw[:, 0:128], lhsT=ident[:], rhs=ident[:],
                                 start=True, stop=True)

            def emit_qkv_half(b, ft, np2, xp_t, dst):
                # half of one of Q'/K'/V' (2 of 4 token chunks); same-weight
                # matmuls back-to-back so LDWEIGHTS amortizes over 2 chunks.
                with nc.named_scope("qkv"):
                    ps = psA.tile([128, 2 * QC], f32, tag="ps")
                    for kc in range(DKC):
                        for half in range(2):
                            ntk = 2 * np2 + half
                            nc.tensor.matmul(
                                ps[:, QC * half:QC * (half + 1)],
                                lhsT=w3_t[kc][:, 128 * ft:128 * (ft + 1)],
                                rhs=xp_t[kc][:, QC * ntk:QC * (ntk + 1)],
                                start=(kc == 0), stop=(kc == DKC - 1),
                            )
                    nc.scalar.copy(
                        dst[:, 2 * QC * np2:2 * QC * (np2 + 1)], ps[:])

            def emit_vtrans_group(b, vp, g, vaug_t):
                # V' -> token-major V for token blocks 4g..4g+3, layout
                # [ones|v_h0|ones|v_h1] so each head's 128-wide PV lhsT slice
                # is [ones(64) | v_h(64)] and the denominator replicas land on
                # PSUM partitions 0-63 (the HW custom-DVE reciprocal reads
                # partition base 0). Groups are interleaved with normal
                # matmuls: transpose-mode doesn't count as PE activity for
                # the HAM clock gate.
                with nc.named_scope("vtrans"):
                    for tk in range(4 * g, 4 * g + 4):
                        ps = psY.tile([128, FPC], bf16, tag="psy")
                        nc.tensor.transpose(
                            ps[:], vp[:, 128 * tk:128 * (tk + 1)], ident[:]
                        )
                        va = vpool.tile([128, 4 * HD], bf16, tag="vaug")
                        nc.vector.tensor_copy(va[:, HD:2 * HD], ps[:, 0:HD])
                        nc.vector.tensor_copy(va[:, 3 * HD:4 * HD], ps[:, HD:2 * HD])
                        nc.gpsimd.memset(va[:, 0:HD], 1.0)
                        nc.gpsimd.memset(va[:, 2 * HD:3 * HD], 1.0)
                        vaug_t.append(va)

            def score_quad(qc, kb2, pts, qp, kp):
                # scores for key blocks (kbA, kbB) x one 512-query chunk for
                # BOTH heads; matmuls alternate PE row groups 0-63/64-127 so
                # T0/T8 execute concurrently. One exp per head covers the
                # whole live range (the dead gap is never read by PV).
                kbA, kbB = 2 * kb2, 2 * kb2 + 1
                j0A = max(0, KB * (kbA - qc * 4))
                j0B = max(0, KB * (kbB - qc * 4))
                pss = [psA.tile([128, 2 * QC], f32, tag="ps", name=f"pss{h}")
                       for h in range(2)]
                for off, kb, j0 in ((0, kbA, j0A), (QC, kbB, j0B)):
                    for h in range(2):
                        nc.tensor.matmul(
                            pss[h][:, off + j0:off + QC],
                            lhsT=kp[HD * h:HD * (h + 1), KB * kb:KB * (kb + 1)],
                            rhs=qp[HD * h:HD * (h + 1), QC * qc + j0:QC * (qc + 1)],
                            start=True, stop=True,
                        )
                for h in range(2):
                    pt = ptpool.tile([KB, 2 * QC], bf16, tag="pt")
                    nc.scalar.activation(pt[:, j0A:2 * QC], pss[h][:, j0A:2 * QC],
                                         EXP, scale=1.0 / NORM)
                    for off, kb, j0 in ((0, kbA, j0A), (QC, kbB, j0B)):
                        p = kb - qc * 4
                        if p >= 0:
                            nc.vector.tensor_tensor(
                                pt[:, off + j0:off + QC],
                                pt[:, off + j0:off + QC],
                                mask_t[p][:, j0:QC],
                                op=AluOpType.mult,
                            )
                        pts[qc, h, kb] = (pt, off, j0)

            def emit_scores(qc, pts, qp, kp):
                nkb = (qc + 1) * (QC // KB)
                with nc.named_scope("score"):
                    for kb2 in range(nkb // 2):
                        score_quad(qc, kb2, pts, qp, kp)

            def emit_pv(qc, pts, vaug_t, ys):
                # PV with denominator replicas; normalize in-place on DVE.
                nkb = (qc + 1) * (QC // KB)
                kb_order = [kb for kb in range(nkb) if kb < qc * 4] + \
                           [kb for kb in range(nkb) if kb >= qc * 4]
                y = ypool.tile([FPC, QC], bf16, tag="y")
                with nc.named_scope("pv"):
                    for h in range(HEADS_PER_CORE):
                        psy = psY.tile([128, QC], f32, tag="psy")
                        for i, kb in enumerate(kb_order):
                            pt, off, j0 = pts[qc, h, kb]
                            lo = 2 * HD * h
                            nc.tensor.matmul(
                                psy[:, j0:QC],
                                lhsT=vaug_t[kb][:, lo:lo + 128],
                                rhs=pt[:, off + j0:off + QC],
                                start=(i == 0), stop=(i == nkb - 1),
                            )
                        # rows 0-63 = denom replicas, 64-127 = y for both
                        # heads. DVE reads at most one PSUM operand per
                        # instruction, so the reciprocal lands in SBUF.
                        rec = recpool.tile([64, QC], f32, tag="rec")
                        nc.vector.reciprocal_approx_fast(
                            rec[:], psy[0:64, :])
                        nc.vector.tensor_tensor(
                            y[HD * h:HD * (h + 1), :],
                            psy[64:128, :],
                            rec[:],
                            op=AluOpType.mult,
                        )
                ys[qc] = y

            def emit_proj(b, qc, ys):
                # all 8 w_proj row-tiles stage into one [128, 8*QC] buffer,
                # shipped by a single 3D-AP DMA (dram side iterated
                # (p, mt, c) with strides (T, 128T, 1)) — 1 descriptor per
                # chunk instead of 8 cuts the Sync-engine issue cost and the
                # drain tail.
                with nc.named_scope("proj"):
                    y = ys[qc]
                    ot = otpool.tile([128, 8 * QC], bf16, tag="ot")
                    for mt in range(D // 128):
                        pso = psO.tile([128, QC], f32, tag="pso")
                        nc.tensor.matmul(
                            pso[:],
                            lhsT=wp_t[:, 128 * mt:128 * (mt + 1)],
                            rhs=y[:],
                            start=True, stop=True,
                        )
                        nc.vector.tensor_copy(
                            ot[:, QC * mt:QC * (mt + 1)], pso[:])
                    dst = out[b].rearrange("(mt p) t -> p mt t", mt=8)
                    dst = dst[:, :, QC * qc:QC * (qc + 1)]
                    src = ot[:].rearrange("p (mt c) -> p mt c", mt=8)
                    nc.sync.dma_start(dst, src)

            def emit_xload(b):
                xp_t = []
                for kc in range(DKC):
                    t = xpool.tile([128, T], bf16, tag="xp")
                    nc.sync.dma_start(t[:], xt[b, kc * 128:(kc + 1) * 128, :])
                    xp_t.append(t)
                return xp_t

            # Software-pipelined emission: QKV/vtrans of batch b+1 are
            # interleaved into batch b's attention so the static PE stream
            # always has dependency-free matmuls to run while ACT works
            # through the exps. The last batch runs its query chunks in
            # reverse so the drain tail ends on the shortest chunk.
            st = [dict() for _ in range(B)]
            st[0]['xp'] = emit_xload(0)
            st[0]['qkv'] = tuple(
                qkpool.tile([128, T], bf16, tag=t, name=f"{t}0")
                for t in ("qp", "kp", "vp"))
            for ft in range(3):
                for np2 in range(2):
                    emit_qkv_half(0, ft, np2, st[0]['xp'], st[0]['qkv'][ft])
            st[0]['vaug'] = []

            for b in range(B):
                s = st[b]
                s.setdefault('pts', {})
                s.setdefault('ys', {})
                s.setdefault('exp', [])
                qp, kp = s['qkv'][0], s['qkv'][1]
                nxt = st[b + 1] if b + 1 < B else None
                if nxt is not None:
                    nxt['xp'] = emit_xload(b + 1)
                    nxt['qkv'] = tuple(
                        qkpool.tile([128, T], bf16, tag=t, name=f"{t}{b + 1}")
                        for t in ("qp", "kp", "vp"))
                _qkv_units = [(ft, np2) for ft in range(3) for np2 in range(2)]

                def qkv1(u):
                    if nxt is not None and u < len(_qkv_units):
                        ft, np2 = _qkv_units[u]
                        emit_qkv_half(b + 1, ft, np2, nxt['xp'], nxt['qkv'][ft])

                def vt(g):
                    # own batch's V transposes, interleaved between dense
                    # matmul stretches (transpose-mode doesn't feed HAM)
                    emit_vtrans_group(b, s['qkv'][2], g, s['vaug'])

                if nxt is not None:
                    nxt['vaug'] = []

                last = b == B - 1
                if not last:
                    sc_order = [qc for qc in range(NQC) if (b, qc) != (B - 1, 3)]
                    vt(0)
                    emit_scores(0, s['pts'], qp, kp)
                    vt(1)
                    qkv1(0)
                    emit_scores(1, s['pts'], qp, kp)
                    vt(2)
                    emit_pv(0, s['pts'], s['vaug'], s['ys'])
                    qkv1(1)
                    emit_scores(2, s['pts'], qp, kp)
                    vt(3)
                    emit_pv(1, s['pts'], s['vaug'], s['ys'])
                    emit_proj(b, 0, s['ys'])
                    qkv1(2)
                    emit_scores(3, s['pts'], qp, kp)
                    emit_pv(2, s['pts'], s['vaug'], s['ys'])
                    emit_proj(b, 1, s['ys'])
                    qkv1(3)
                    emit_pv(3, s['pts'], s['vaug'], s['ys'])
                    emit_proj(b, 2, s['ys'])
                    qkv1(4)
                    emit_proj(b, 3, s['ys'])
                    qkv1(5)
                    if b + 1 == B - 1:
                        # hoist the last batch's qc3 scores behind its QKV
                        nxt.setdefault('pts', {})
                        emit_scores(3, nxt['pts'], nxt['qkv'][0], nxt['qkv'][1])
                else:
                    # qc3's scores were hoisted into b-1's schedule; compute
                    # all remaining scores up-front (interleaved with the V
                    # transposes), then drain dense PV+proj largest-first.
                    vt(0)
                    emit_scores(2, s['pts'], qp, kp)
                    vt(1)
                    emit_scores(1, s['pts'], qp, kp)
                    vt(2)
                    emit_scores(0, s['pts'], qp, kp)
                    vt(3)
                    emit_pv(3, s['pts'], s['vaug'], s['ys'])
                    emit_pv(2, s['pts'], s['vaug'], s['ys'])
                    emit_proj(b, 3, s['ys'])
                    emit_pv(1, s['pts'], s['vaug'], s['ys'])
                    emit_proj(b, 2, s['ys'])
                    emit_pv(0, s['pts'], s['vaug'], s['ys'])
                    emit_proj(b, 1, s['ys'])
                    emit_proj(b, 0, s['ys'])

    nc.compile()
    return nc


def _get_nc():
    if "nc" not in _cache:
        _cache["nc"] = _build()
    return _cache["nc"]


def _make_masks():
    i = np.arange(KB)[:, None]
    j = np.arange(QC)[None, :]
    m = np.zeros((4, KB, QC), dtype=np.float32)
    for p in range(4):
        m[p] = (j >= (KB * p + i)).astype(np.float32)
    return m.astype(_BF16)


def shard_inputs(x, w_qkv, w_proj):
    xt = np.ascontiguousarray(np.asarray(x, dtype=np.float32).transpose(0, 2, 1))
    xt = xt.astype(_BF16)
    w_qkv = np.asarray(w_qkv, dtype=np.float32)
    w_proj = np.asarray(w_proj, dtype=np.float32)
    masks = _make_masks()
    in_maps = []
    for c in range(N_CORES):
        qcols = slice(FPC * c, FPC * (c + 1))
        kcols = slice(D + FPC * c, D + FPC * (c + 1))
        vcols = slice(2 * D + FPC * c, 2 * D + FPC * (c + 1))
        w3_c = np.concatenate(
            [w_qkv[:, qcols], w_qkv[:, kcols], w_qkv[:, vcols]], axis=1)
        in_maps.append({
            "xt": xt,
            "w3": np.ascontiguousarray(w3_c).astype(_BF16),
            "wp": np.ascontiguousarray(w_proj[FPC * c:FPC * (c + 1), :]).astype(_BF16),
            "masks": masks,
        })
    return in_maps


def unshard(results):
    total = results[0]["out"].astype(np.float32)
    for r in results[1:]:
        total += r["out"].astype(np.float32)
    return np.ascontiguousarray(total.transpose(0, 2, 1))


def run(inputs, trace=False, **kw):
    from concourse.bass_utils import run_bass_kernel_spmd

    nc = _get_nc()
    in_maps = shard_inputs(inputs["x"], inputs["w_qkv"], inputs["w_proj"])
    res = run_bass_kernel_spmd(nc, in_maps, core_ids=list(range(N_CORES)),
                               trace=trace, **kw)
    return unshard(res.results), res


def kernel(**inputs):
    out, _ = run(inputs, trace=False)
    return out


# revision 27
# speedup vs baseline: 1.1875x; 1.1875x over previous
"""Multi-head causal attention (B=4, T=2048, D=1024, H=16) on 8 TRN2 cores.

Tensor-parallel over heads: core c computes heads {2c, 2c+1}. Each core:
  - Q', K', V' feature-major ([feat, tok]) via 512-wide PE matmuls,
  - V' -> token-major V via PE transposes, stored as [ones|v_h0|ones|v_h1] so
    each head's 128-wide PV lhsT slice is [ones(64) | v_h(64)]: the ones
    columns replicate the softmax denominator onto PSUM partitions 0-63
    (the HW custom-DVE reciprocal reads partition base 0 regardless of the
    AP offset, so the denominator must live there),
  - S^T = K'^T Q' tiles [128 k x 512 q], h0/h1 matmuls alternated so the
    64-deep score matmuls run concurrently on PE row groups 0-63/64-127;
    one exp covers each pair's live range (dead gap is never read by PV),
  - exp (no max subtraction: |S|/32 <= ~2), multiplicative 0/1 causal mask
    on diagonal blocks, narrowed to their live query range,
  - PV accumulates [denom replicas | y_h] per psum bank; normalization is a
    DVE reciprocal_approx_fast into SBUF + one partition-offset multiply,
  - w_proj row-slice partial matmuls per query chunk, lag-1 behind PV,
  - partial projections written bf16; host sums the 8 cores and transposes.

Scheduling: the whole emission is software-pipelined. The next batch's QKV
is split into six half-sweeps interleaved into the current batch's attention
(filler for the PE while ACT drains exps — keeps the HAM clock gate at
2.4 GHz); V transposes go in groups of four between dense stretches
(transpose-mode doesn't count as PE activity for HAM); the last batch's
largest score chunk is hoisted into the previous batch and its drain runs
largest-first so the tail ends on the smallest chunk. 250 warmup matmuls
cover the initial x DMA.
"""

import sys

for _p in ("/opt/trn_rl_repo",):
    if _p not in sys.path:
        sys.path.append(_p)

import numpy as np
import ml_dtypes

B, T, D = 4, 2048, 1024
H = 16
HD = D // H
NORM = float(np.sqrt(D))
N_CORES = 8
HEADS_PER_CORE = H // N_CORES          # 2
FPC = HEADS_PER_CORE * HD              # 128 features per core
QC = 512                               # query chunk
NQC = T // QC                          # 4
KB = 128                               # key block
DKC = D // 128                         # 8 contraction chunks over D

_BF16 = ml_dtypes.bfloat16

_cache = {}


def _build():
    import concourse.bacc as bacc
    import concourse.mybir as mybir
    from concourse.tile import TileContext
    from concourse.alu_op_type import AluOpType
    from concourse.masks import make_identity

    f32 = mybir.dt.float32
    bf16 = mybir.dt.bfloat16
    EXP = mybir.ActivationFunctionType.Exp

    nc = bacc.Bacc("TRN2", target_bir_lowering=False, debug=False,
                   num_devices=N_CORES)

    xt = nc.dram_tensor("xt", [B, D, T], bf16, kind="ExternalInput").ap()
    w3 = nc.dram_tensor("w3", [D, 3 * FPC], bf16, kind="ExternalInput").ap()
    wp = nc.dram_tensor("wp", [FPC, D], bf16, kind="ExternalInput").ap()
    masks = nc.dram_tensor("masks", [4, KB, QC], bf16, kind="ExternalInput").ap()
    out = nc.dram_tensor("out", [B, D, T], bf16, kind="ExternalOutput").ap()

    with TileContext(nc) as tc:
        with (
            tc.tile_pool(name="const", bufs=1) as cpool,
            tc.tile_pool(name="xp", bufs=10) as xpool,
            tc.tile_pool(name="qk", bufs=2) as qkpool,
            tc.tile_pool(name="vaug", bufs=24) as vpool,
            tc.tile_pool(name="pt", bufs=40) as ptpool,
            tc.tile_pool(name="y", bufs=4) as ypool,
            tc.tile_pool(name="ot", bufs=6) as otpool,
            tc.tile_pool(name="rec", bufs=2) as recpool,
            tc.tile_pool(name="psA", bufs=2, space="PSUM") as psA,
            tc.tile_pool(name="psY", bufs=2, space="PSUM") as psY,
            tc.tile_pool(name="psO", bufs=2, space="PSUM") as psO,
        ):
            # ---- constants ----
            w3_t = []
            for kc in range(DKC):
                t = cpool.tile([128, 3 * FPC], bf16, tag=f"w3{kc}")
                nc.sync.dma_start(t[:], w3[kc * 128:(kc + 1) * 128, :])
                w3_t.append(t)
            wp_t = cpool.tile([FPC, D], bf16, tag="wp")
            nc.sync.dma_start(wp_t[:], wp[:])
            mask_t = []
            for p in range(4):
                t = cpool.tile([KB, QC], bf16, tag=f"mask{p}")
                nc.sync.dma_start(t[:], masks[p])
                mask_t.append(t)
            ident = cpool.tile([128, 128], bf16, tag="ident")
            make_identity(nc, ident[:])

            # PE warmup during the initial x DMA: keeps the HAM clock-gate
            # busy so real matmuls start at 2.4 GHz.
            psw = psO.tile([128, QC], f32, tag="pso")
            for _ in range(250):
                nc.tensor.matmul(psw[:, 0:128], lhsT=ident[:], rhs=ident[:],
                                 start=True, stop=True)

            def emit_qkv_half(b, ft, np2, xp_t, dst):
                # half of one of Q'/K'/V' (2 of 4 token chunks); same-weight
                # matmuls back-to-back so LDWEIGHTS amortizes over 2 chunks.
                with nc.named_scope("qkv"):
                    ps = psA.tile([128, 2 * QC], f32, tag="ps")
                    for kc in range(DKC):
                        for half in range(2):
                            ntk = 2 * np2 + half
                            nc.tensor.matmul(
                                ps[:, QC * half:QC * (half + 1)],
                                lhsT=w3_t[kc][:, 128 * ft:128 * (ft + 1)],
                                rhs=xp_t[kc][:, QC * ntk:QC * (ntk + 1)],
                                start=(kc == 0), stop=(kc == DKC - 1),
                            )
                    nc.scalar.copy(
                        dst[:, 2 * QC * np2:2 * QC * (np2 + 1)], ps[:])

            def emit_vtrans_group(b, vp, g, vaug_t):
                # V' -> token-major V for token blocks 4g..4g+3, layout
                # [ones|v_h0|ones|v_h1] so each head's 128-wide PV lhsT slice
                # is [ones(64) | v_h(64)] and the denominator replicas land on
                # PSUM partitions 0-63 (the HW custom-DVE reciprocal reads
                # partition base 0). Groups are interleaved with normal
                # matmuls: transpose-mode doesn't count as PE activity for
                # the HAM clock gate.
                with nc.named_scope("vtrans"):
                    for tk in range(4 * g, 4 * g + 4):
                        ps = psY.tile([128, FPC], bf16, tag="psy")
                        nc.tensor.transpose(
                            ps[:], vp[:, 128 * tk:128 * (tk + 1)], ident[:]
                        )
                        va = vpool.tile([128, 4 * HD], bf16, tag="vaug")
                        nc.vector.tensor_copy(va[:, HD:2 * HD], ps[:, 0:HD])
                        nc.vector.tensor_copy(va[:, 3 * HD:4 * HD], ps[:, HD:2 * HD])
                        nc.gpsimd.memset(va[:, 0:HD], 1.0)
                        nc.gpsimd.memset(va[:, 2 * HD:3 * HD], 1.0)
                        vaug_t.append(va)

            def score_quad(qc, kb2, pts, qp, kp):
                # scores for key blocks (kbA, kbB) x one 512-query chunk for
                # BOTH heads; matmuls alternate PE row groups 0-63/64-127 so
                # T0/T8 execute concurrently. One exp per head covers the
                # whole live range (the dead gap is never read by PV).
                kbA, kbB = 2 * kb2, 2 * kb2 + 1
                j0A = max(0, KB * (kbA - qc * 4))
                j0B = max(0, KB * (kbB - qc * 4))
                pss = [psA.tile([128, 2 * QC], f32, tag="ps", name=f"pss{h}")
                       for h in range(2)]
                for off, kb, j0 in ((0, kbA, j0A), (QC, kbB, j0B)):
                    for h in range(2):
                        nc.tensor.matmul(
                            pss[h][:, off + j0:off + QC],
                            lhsT=kp[HD * h:HD * (h + 1), KB * kb:KB * (kb + 1)],
                            rhs=qp[HD * h:HD * (h + 1), QC * qc + j0:QC * (qc + 1)],
                            start=True, stop=True,
                        )
                for h in range(2):
                    pt = ptpool.tile([KB, 2 * QC], bf16, tag="pt")
                    nc.scalar.activation(pt[:, j0A:2 * QC], pss[h][:, j0A:2 * QC],
                                         EXP, scale=1.0 / NORM)
                    for off, kb, j0 in ((0, kbA, j0A), (QC, kbB, j0B)):
                        p = kb - qc * 4
                        if p >= 0:
                            nc.vector.tensor_tensor(
                                pt[:, off + j0:off + QC],
                                pt[:, off + j0:off + QC],
                                mask_t[p][:, j0:QC],
                                op=AluOpType.mult,
                            )
                        pts[qc, h, kb] = (pt, off, j0)

            def emit_scores(qc, pts, qp, kp):
                nkb = (qc + 1) * (QC // KB)
                with nc.named_scope("score"):
                    for kb2 in range(nkb // 2):
                        score_quad(qc, kb2, pts, qp, kp)

            def emit_pv(qc, pts, vaug_t, ys):
                # PV with denominator replicas; normalize in-place on DVE.
                nkb = (qc + 1) * (QC // KB)
                kb_order = [kb for kb in range(nkb) if kb < qc * 4] + \
                           [kb for kb in range(nkb) if kb >= qc * 4]
                y = ypool.tile([FPC, QC], bf16, tag="y")
                with nc.named_scope("pv"):
                    for h in range(HEADS_PER_CORE):
                        psy = psY.tile([128, QC], f32, tag="psy")
                        for i, kb in enumerate(kb_order):
                            pt, off, j0 = pts[qc, h, kb]
                            lo = 2 * HD * h
                            nc.tensor.matmul(
                                psy[:, j0:QC],
                                lhsT=vaug_t[kb][:, lo:lo + 128],
                                rhs=pt[:, off + j0:off + QC],
                                start=(i == 0), stop=(i == nkb - 1),
                            )
                        # rows 0-63 = denom replicas, 64-127 = y for both
                        # heads. DVE reads at most one PSUM operand per
                        # instruction, so the reciprocal lands in SBUF.
                        rec = recpool.tile([64, QC], f32, tag="rec")
                        nc.vector.reciprocal_approx_fast(
                            rec[:], psy[0:64, :])
                        nc.vector.tensor_tensor(
                            y[HD * h:HD * (h + 1), :],
                            psy[64:128, :],
                            rec[:],
                            op=AluOpType.mult,
                        )
                ys[qc] = y

            def emit_proj(b, qc, ys):
                with nc.named_scope("proj"):
                    y = ys[qc]
                    for mt in range(D // 128):
                        pso = psO.tile([128, QC], f32, tag="pso")
                        nc.tensor.matmul(
                            pso[:],
                            lhsT=wp_t[:, 128 * mt:128 * (mt + 1)],
                            rhs=y[:],
                            start=True, stop=True,
                        )
                        ot = otpool.tile([128, QC], bf16, tag="ot")
                        nc.vector.tensor_copy(ot[:], pso[:])
                        nc.sync.dma_start(
                            out[b, 128 * mt:128 * (mt + 1), QC * qc:QC * (qc + 1)],
                            ot[:],
                        )

            def emit_xload(b):
                xp_t = []
                for kc in range(DKC):
                    t = xpool.tile([128, T], bf16, tag="xp")
                    nc.sync.dma_start(t[:], xt[b, kc * 128:(kc + 1) * 128, :])
                    xp_t.append(t)
                return xp_t

            # Software-pipelined emission: QKV/vtrans of batch b+1 are
            # interleaved into batch b's attention so the static PE stream
            # always has dependency-free matmuls to run while ACT works
            # through the exps. The last batch runs its query chunks in
            # reverse so the drain tail ends on the shortest chunk.
            st = [dict() for _ in range(B)]
            st[0]['xp'] = emit_xload(0)
            st[0]['qkv'] = tuple(
                qkpool.tile([128, T], bf16, tag=t, name=f"{t}0")
                for t in ("qp", "kp", "vp"))
            for ft in range(3):
                for np2 in range(2):
                    emit_qkv_half(0, ft, np2, st[0]['xp'], st[0]['qkv'][ft])
            st[0]['vaug'] = []

            for b in range(B):
                s = st[b]
                s.setdefault('pts', {})
                s.setdefault('ys', {})
                s.setdefault('exp', [])
                qp, kp = s['qkv'][0], s['qkv'][1]
                nxt = st[b + 1] if b + 1 < B else None
                if nxt is not None:
                    nxt['xp'] = emit_xload(b + 1)
                    nxt['qkv'] = tuple(
                        qkpool.tile([128, T], bf16, tag=t, name=f"{t}{b + 1}")
                        for t in ("qp", "kp", "vp"))
                _qkv_units = [(ft, np2) for ft in range(3) for np2 in range(2)]

                def qkv1(u):
                    if nxt is not None and u < len(_qkv_units):
                        ft, np2 = _qkv_units[u]
                        emit_qkv_half(b + 1, ft, np2, nxt['xp'], nxt['qkv'][ft])

                def vt(g):
                    # own batch's V transposes, interleaved between dense
                    # matmul stretches (transpose-mode doesn't feed HAM)
                    emit_vtrans_group(b, s['qkv'][2], g, s['vaug'])

                if nxt is not None:
                    nxt['vaug'] = []

                last = b == B - 1
                if not last:
                    sc_order = [qc for qc in range(NQC) if (b, qc) != (B - 1, 3)]
                    vt(0)
                    emit_scores(0, s['pts'], qp, kp)
                    vt(1)
                    qkv1(0)
                    emit_scores(1, s['pts'], qp, kp)
                    vt(2)
                    emit_pv(0, s['pts'], s['vaug'], s['ys'])
                    qkv1(1)
                    emit_scores(2, s['pts'], qp, kp)
                    vt(3)
                    emit_pv(1, s['pts'], s['vaug'], s['ys'])
                    emit_proj(b, 0, s['ys'])
                    qkv1(2)
                    emit_scores(3, s['pts'], qp, kp)
                    emit_pv(2, s['pts'], s['vaug'], s['ys'])
                    emit_proj(b, 1, s['ys'])
                    qkv1(3)
                    emit_pv(3, s['pts'], s['vaug'], s['ys'])
                    emit_proj(b, 2, s['ys'])
                    qkv1(4)
                    emit_proj(b, 3, s['ys'])
                    qkv1(5)
                    if b + 1 == B - 1:
                        # hoist the last batch's qc3 scores behind its QKV
                        nxt.setdefault('pts', {})
                        emit_scores(3, nxt['pts'], nxt['qkv'][0], nxt['qkv'][1])
                else:
                    # qc3's scores were hoisted into b-1's schedule; compute
                    # all remaining scores up-front (interleaved with the V
                    # transposes), then drain dense PV+proj largest-first.
                    vt(0)
                    emit_scores(2, s['pts'], qp, kp)
                    vt(1)
                    emit_scores(1, s['pts'], qp, kp)
                    vt(2)
                    emit_scores(0, s['pts'], qp, kp)
                    vt(3)
                    emit_pv(3, s['pts'], s['vaug'], s['ys'])
                    emit_pv(2, s['pts'], s['vaug'], s['ys'])
                    emit_proj(b, 3, s['ys'])
                    emit_pv(1, s['pts'], s['vaug'], s['ys'])
                    emit_proj(b, 2, s['ys'])
                    emit_pv(0, s['pts'], s['vaug'], s['ys'])
                    emit_proj(b, 1, s['ys'])
                    emit_proj(b, 0, s['ys'])

    nc.compile()
    return nc


def _get_nc():
    if "nc" not in _cache:
        _cache["nc"] = _build()
    return _cache["nc"]


def _make_masks():
    i = np.arange(KB)[:, None]
    j = np.arange(QC)[None, :]
    m = np.zeros((4, KB, QC), dtype=np.float32)
    for p in range(4):
        m[p] = (j >= (KB * p + i)).astype(np.float32)
    return m.astype(_BF16)


def shard_inputs(x, w_qkv, w_proj):
    xt = np.ascontiguousarray(np.asarray(x, dtype=np.float32).transpose(0, 2, 1))
    xt = xt.astype(_BF16)
    w_qkv = np.asarray(w_qkv, dtype=np.float32)
    w_proj = np.asarray(w_proj, dtype=np.float32)
    masks = _make_masks()
    in_maps = []
    for c in range(N_CORES):
        qcols = slice(FPC * c, FPC * (c + 1))
        kcols = slice(D + FPC * c, D + FPC * (c + 1))
        vcols = slice(2 * D + FPC * c, 2 * D + FPC * (c + 1))
        w3_c = np.concatenate(
            [w_qkv[:, qcols], w_qkv[:, kcols], w_qkv[:, vcols]], axis=1)
        in_maps.append({
            "xt": xt,
            "w3": np.ascontiguousarray(w3_c).astype(_BF16),
            "wp": np.ascontiguousarray(w_proj[FPC * c:FPC * (c + 1), :]).astype(_BF16),
            "masks": masks,
        })
    return in_maps


def unshard(results):
    total = results[0]["out"].astype(np.float32)
    for r in results[1:]:
        total += r["out"].astype(np.float32)
    return np.ascontiguousarray(total.transpose(0, 2, 1))


def run(inputs, trace=False, **kw):
    from concourse.bass_utils import run_bass_kernel_spmd

    nc = _get_nc()
    in_maps = shard_inputs(inputs["x"], inputs["w_qkv"], inputs["w_proj"])
    res = run_bass_kernel_spmd(nc, in_maps, core_ids=list(range(N_CORES)),
                               trace=trace, **kw)
    return unshard(res.results), res


def kernel(**inputs):
    out, _ = run(inputs, trace=False)
    return out
